# revision 14
# baseline (speedup 1.0000x reference)
"""Multi-head self-attention (16 heads, fake-quantized projections) on 8 trn2 cores.

Sharding: core c handles batch b = c // 4 and head group hg = c % 4 (global
heads 4*hg .. 4*hg+3). Each core computes its 4 heads' attention and a partial
output projection [S, E]; the host sums the 4 partials per batch.

v2 pipeline (vs the v1 baseline at ~445us):
  - bf16 for x / w_qkv / w_v / w_out / q / k (fp32r kept for p and v in the
    PV path), halving DMA + SBUF traffic and enabling fast weight loads.
  - streamed QKV: x^T chunks of 512 positions are DMA'd, projected, RoPE'd
    and rearranged while the next chunk loads - no dead start.
  - attention: scores run as row-tiled K=64 matmul pairs (concurrent on the
    PE via base-partition tile_position); exp is split between the ACT
    engine (exact) and the DVE (Schraudolph bitcast exp, ~3% max rel err)
    to break the single-engine softmax bottleneck; PV is skewed 4 kt behind
    scores so the in-order PE queue never stalls.
  - softmax normalization: evict U to SBUF, reciprocal_approx_fast (DVE),
    gpsimd partition_broadcast, fused multiply - no slow DVE reciprocal and
    no PE broadcast matmuls.
  - output projection: u restacked into head pairs (K=128 matmuls), psum
    accumulated over both pairs and DMA'd f32 straight from PSUM to DRAM.
"""
import sys, types
import numpy as np

sys.path.insert(0, "/opt/trn_rl_repo")

# NTFF profile hook shim (stub antenv package lacks axon_hooks; harmless if absent)
try:
    from trn_agent_boot.trn_boot import _ntff_profile_via_ctypes
    _hook = _ntff_profile_via_ctypes("/opt/axon/libaxon_pjrt.so")
    _m = types.ModuleType("antenv.axon_hooks")
    _m.get_axon_ntff_profile_hook = lambda: _hook
    _m.set_axon_ntff_profile_hook = lambda h: None
    sys.modules.setdefault("antenv.axon_hooks", _m)
except Exception:
    pass

import ml_dtypes
import concourse.bacc as bacc
import concourse.tile as tile
from concourse import mybir
from concourse import bass_utils as _bu
_bu.upload_artifacts = lambda tmpdir: "local://" + tmpdir

F32 = mybir.dt.float32
F32R = mybir.dt.float32r
BF16 = mybir.dt.bfloat16
I16 = mybir.dt.int16
AF = mybir.ActivationFunctionType
ALU = mybir.AluOpType

B, S, E = 2, 2048, 1024
H, D = 16, 64
HL = 4          # heads per core
ET = E // 128   # 8 e-tiles
CH = 512        # projection s-chunk
NCH = S // CH
KT = S // 128   # 16 kpos tiles
QC = 512        # attention q-chunk
NQ = S // QC
SKEW = 4        # PV trails scores by this many kt iterations

# Schraudolph fast exp in bf16 bit space:
# exp(s/8) ~= bitcast_bf16(int16(EXPA*s + EXPB)); max rel err 3.3e-2, |s|<=32.
EXPA = float(np.float32(2.0 ** 23 * 0.125 / np.log(2.0) / 65536.0))
EXPB = float(np.float32((127 << 7) - 5.6))


def exp_on_dve(qc, kt, pr):
    """Which score tiles get the DVE bitcast exp (vs exact ACT exp)."""
    return (kt * 2 + pr) % 5 >= 3


def quantize_bits_np(x):
    """Exact numpy replica of reference.quantize_bits(x, 8) in float32."""
    x = np.asarray(x, dtype=np.float32)
    qmax = np.float32(255.0)
    x_min = x.min()
    x_max = x.max()
    scale = np.float32((x_max - x_min) / np.float32(qmax + np.float32(1e-8)))
    x_q = np.round(np.clip((x - x_min) / np.float32(scale + np.float32(1e-8)),
                           np.float32(0.0), qmax)).astype(np.float32)
    return x_q * scale + x_min


def rope_tables():
    inv_freq = (1.0 / 10000.0 ** (np.arange(0, D, 2, dtype=np.float32) / D)).astype(np.float32)
    t = np.arange(S, dtype=np.float32)
    freqs = t[:, None].astype(np.float32) * inv_freq[None, :]
    sin = np.sin(freqs).astype(np.float32)   # (S, 32)
    cos = np.cos(freqs).astype(np.float32)
    cosT = np.tile(np.ascontiguousarray(cos.T), (4, 1))  # (128, S), 32-row period
    sinT = np.tile(np.ascontiguousarray(sin.T), (4, 1))
    return cosT, sinT


def build_kernel(debug=False):
    nc = bacc.Bacc(trn_type="TRN2")
    dbg = {}
    if debug:
        for name, shape, dt in [
                ("d_qc0", [128, S], BF16), ("d_kc0", [128, S], BF16),
                ("d_p_act", [128, 1024], BF16), ("d_p_dve", [128, 1024], BF16),
                ("d_uraw0", [D + 1, 512], F32), ("d_den0", [1, 512], F32),
                ("d_bc0", [D, 512], F32), ("d_up0", [128, S], BF16),
                ("d_va0", [128, HL * (D + 1)], BF16)]:
            dbg[name] = nc.declare_dram_parameter(name, shape, dt, isOutput=True)
    xt = nc.declare_dram_parameter("xt", [128, ET, S], BF16, isOutput=False)
    wqk = nc.declare_dram_parameter("wqk", [128, 4, ET, 128], BF16, isOutput=False)
    wv = nc.declare_dram_parameter("wv", [128, ET, HL * D], BF16, isOutput=False)
    wo = nc.declare_dram_parameter("wo", [128, 2, E], BF16, isOutput=False)
    cost = nc.declare_dram_parameter("cost", [128, S], F32, isOutput=False)
    sint = nc.declare_dram_parameter("sint", [128, S], F32, isOutput=False)
    # ypart[g, p, i*E + e] = y[g*512 + i*128 + p, e]
    ypart = nc.declare_dram_parameter("ypart", [4, 128, 4 * E], BF16, isOutput=True)

    from concourse.tile_rust import add_dep_helper

    with tile.TileContext(nc) as tc:
        with (
            tc.tile_pool(name="sb", bufs=1) as sb,
            tc.tile_pool(name="ps", bufs=2, space="PSUM") as ps,
        ):
            # ---------------- weights + tables (single big DMAs)
            # order matters: first-chunk deps (xT0, wv, wqk) lead the queue
            xT = []
            t0 = sb.tile([128, ET, CH], BF16, tag="xT", bufs=2, name="xT0")
            nc.sync.dma_start(out=t0, in_=xt[:, :, 0:CH])
            xT.append(t0)
            wv_sb = sb.tile([128, ET, HL * D], BF16, tag="wv", bufs=1)
            nc.sync.dma_start(out=wv_sb, in_=wv[:, :, :])
            wqk_sb = sb.tile([128, 4, ET, 128], BF16, tag="wqk", bufs=1)
            nc.sync.dma_start(out=wqk_sb, in_=wqk[:, :, :, :])
            cos_sb = sb.tile([128, S], F32, tag="cos", bufs=1)
            sin_sb = sb.tile([128, S], F32, tag="sin", bufs=1)
            nc.sync.dma_start(out=cos_sb, in_=cost[:, :])
            nc.sync.dma_start(out=sin_sb, in_=sint[:, :])
            wo_sb = sb.tile([128, 2, E], BF16, tag="wo", bufs=1)

            # persistent activation tiles
            # rot layout per pair: rows 32h..32h+32 of rot[half] = head h
            # (lo half / hi half of the rotated dim)
            qrot = [sb.tile([128, S], BF16, tag="rot", bufs=4, name=f"qrot{i}")
                    for i in range(2)]
            krot = [sb.tile([128, S], BF16, tag="rot", bufs=4, name=f"krot{i}")
                    for i in range(2)]
            # head-contiguous q/k: cont[p] rows 0-63 = head 2p, 64-127 = head 2p+1
            qcont = [sb.tile([128, S], BF16, tag="cont", bufs=4, name=f"qcont{p}")
                     for p in range(2)]
            kcont = [sb.tile([128, S], BF16, tag="cont", bufs=4, name=f"kcont{p}")
                     for p in range(2)]
            # normalized attention outputs, head-pair stacked for K=128 out proj
            u_pair = [sb.tile([128, S], BF16, tag="upair", bufs=2, name=f"upair{p}")
                      for p in range(2)]
            v_aug = []

            # ---------------- phase 1: streamed QKV + RoPE
            def rope_pair(pair, rot, ci):
                csl = slice(ci * CH, (ci + 1) * CH)
                bb = ps.tile([128, 1024], F32, tag="sc", bufs=2)
                b1, b2 = bb[:, 0:512], bb[:, 512:1024]
                for et in range(ET):
                    nc.tensor.matmul(b1, wqk_sb[:, pair, et, :], xT[ci][:, et, :],
                                     start=(et == 0), stop=(et == ET - 1))
                for et in range(ET):
                    nc.tensor.matmul(b2, wqk_sb[:, pair + 1, et, :], xT[ci][:, et, :],
                                     start=(et == 0), stop=(et == ET - 1))
                t1 = sb.tile([128, CH], F32, tag="t1", bufs=2)
                t2 = sb.tile([128, CH], F32, tag="t2", bufs=2)
                t3 = sb.tile([128, CH], F32, tag="t3", bufs=2)
                t4 = sb.tile([128, CH], F32, tag="t4", bufs=2)
                nc.vector.tensor_mul(t1, b1, cos_sb[:, csl])
                nc.vector.tensor_mul(t2, b2, sin_sb[:, csl])
                nc.vector.tensor_mul(t3, b1, sin_sb[:, csl])
                nc.vector.tensor_mul(t4, b2, cos_sb[:, csl])
                nc.gpsimd.tensor_sub(rot[0][:, csl], t1, t2)
                nc.gpsimd.tensor_add(rot[1][:, csl], t3, t4)

            def rearrange(rot, cont, sh):
                ssl = slice(sh * 1024, (sh + 1) * 1024)
                for h in range(HL):
                    p, j = divmod(h, 2)
                    for half in range(2):
                        rows_out = slice(64 * j + 32 * half, 64 * j + 32 * half + 32)
                        nc.sync.dma_start(out=cont[p][rows_out, ssl],
                                          in_=rot[half][32 * h:32 * h + 32, ssl])

            for ci in range(NCH):
                if ci + 1 < NCH:
                    t = sb.tile([128, ET, CH], BF16, tag="xT", bufs=2,
                                name=f"xT{ci + 1}")
                    nc.sync.dma_start(out=t, in_=xt[:, :, (ci + 1) * CH:(ci + 2) * CH])
                    xT.append(t)
                # v projection (natural [s, d]) + ones column
                for st_l in range(CH // 128):
                    st = ci * (CH // 128) + st_l
                    pv = ps.tile([128, HL * D], F32, tag="sc", bufs=2)
                    for et in range(ET):
                        nc.tensor.matmul(pv, xT[ci][:, et, st_l * 128:(st_l + 1) * 128],
                                         wv_sb[:, et, :], start=(et == 0),
                                         stop=(et == ET - 1))
                    va = sb.tile([128, HL, D + 1], BF16, tag="vaug", bufs=KT,
                                 name=f"vaug{st}")
                    nc.gpsimd.memset(va, 1.0)
                    nc.scalar.copy(va[:, :, 0:D],
                                   pv.rearrange("p (h d) -> p h d", h=HL))
                    if debug and st == 0:
                        nc.sync.dma_start(
                            out=dbg["d_va0"][:, :],
                            in_=va.rearrange("p a b -> p (a b)"))
                    v_aug.append(va)
                rope_pair(2, krot, ci)
                rope_pair(0, qrot, ci)
                if ci == 0:
                    nc.sync.dma_start(out=wo_sb, in_=wo[:, :, :])
                if ci % 2 == 1:
                    rearrange(krot, kcont, ci // 2)
                    rearrange(qrot, qcont, ci // 2)

            # ---------------- phase 2: attention
            # scores^T [kpos, q] per head; exp on ACT (fused scale 1/8) or DVE
            # (bitcast exp); PV accumulates U^T[d, q] + denominator row over
            # kpos tiles in PSUM, skewed SKEW kts behind the scores.
            p_store = {}      # (qc, kt) -> [p_pr0, p_pr1]
            sup = {}          # (qc, h) -> psum accumulator
            u_raw = {}        # (qc, h) -> evicted [D+1, 512] sbuf tile
            den_sb = {}       # (qc, h) -> denominator row at partition 0
            bc_sb = {}        # (qc, h) -> broadcast reciprocal [D, 512]
            last_pv = [None]
            s_last = [None]

            def emit_scores_exp(qc, kt):
                qsl = slice(qc * QC, (qc + 1) * QC)
                ksl = slice(kt * 128, (kt + 1) * 128)
                pts = []
                for pr in range(2):
                    s_ps = ps.tile([128, 1024], F32, tag="sc", bufs=2)
                    if pr == 0:
                        # keep-alive: throwaway matmul (overwritten by the
                        # start=True scores below) bridges PE idle gaps so the
                        # HAM clock gate stays at 2.4 GHz
                        dm = nc.tensor.matmul(
                            s_ps[:, 0:256], kcont[0][0:64, ksl],
                            qcont[0][0:64, qc * QC:qc * QC + 256],
                            start=True, stop=True, skip_group_check=True)
                        if last_pv[0] is not None:
                            add_dep_helper(dm.ins, last_pv[0].ins, sync=False,
                                           reason="pe order")
                    for j in range(2):
                        mm = nc.tensor.matmul(
                            s_ps[:, j * 512:(j + 1) * 512],
                            kcont[pr][64 * j:64 * j + 64, ksl],
                            qcont[pr][64 * j:64 * j + 64, qsl],
                            start=True, stop=True)
                        if last_pv[0] is not None:
                            add_dep_helper(mm.ins, last_pv[0].ins, sync=False,
                                           reason="pe order")
                        s_last[0] = mm
                    p_t = sb.tile([128, 1024], BF16, tag="p", bufs=2 * (SKEW + 1),
                                  name=f"p{pr}")
                    if exp_on_dve(qc, kt, pr):
                        nc.vector.tensor_scalar(p_t.bitcast(I16), s_ps,
                                                EXPA, EXPB, ALU.mult, ALU.add)
                        if debug and qc == 0 and kt == 0 and pr == 1:
                            nc.sync.dma_start(out=dbg["d_p_dve"][:, :], in_=p_t)
                    else:
                        nc.scalar.activation(p_t, s_ps, AF.Exp, scale=0.125)
                        if debug and qc == 0 and kt == 0 and pr == 0:
                            nc.sync.dma_start(out=dbg["d_p_act"][:, :], in_=p_t)
                    pts.append(p_t)
                p_store[(qc, kt)] = pts

            def emit_pv(qc, kt):
                pts = p_store.pop((qc, kt))
                for h in range(HL):
                    if kt == 0:
                        sup[(qc, h)] = ps.tile([D + 1, 512], F32, tag="pv",
                                               bufs=HL, name=f"u{h}_{qc}")
                    mm = nc.tensor.matmul(
                        sup[(qc, h)], v_aug[kt][:, h, :],
                        pts[h // 2][:, (h % 2) * 512:(h % 2) * 512 + 512],
                        start=(kt == 0), stop=(kt == KT - 1))
                    if s_last[0] is not None:
                        add_dep_helper(mm.ins, s_last[0].ins, sync=False,
                                       reason="pe order")
                    last_pv[0] = mm

            def emit_evict(qc):
                # free the PSUM accumulators fast: 2 copies on ACT, 2 on DVE
                for h in range(HL):
                    t = sb.tile([D + 1, 512], F32, tag="uraw", bufs=4,
                                name=f"uraw{h}_{qc}")
                    if h % 2 == 0:
                        nc.scalar.copy(t, sup[(qc, h)])
                    else:
                        nc.vector.tensor_copy(t, sup[(qc, h)])
                    if debug and qc == 0 and h == 0:
                        nc.sync.dma_start(out=dbg["d_uraw0"][:, :], in_=t)
                    u_raw[(qc, h)] = t
                    del sup[(qc, h)]

            def emit_den_dma(qc):
                # custom DVE/gpsimd ops are partition-0 anchored: move the
                # denominator row (psum partition 64) to a partition-0 tile
                for h in range(HL):
                    r = sb.tile([1, 512], F32, tag="recip", bufs=8, name=f"r{h}_{qc}")
                    nc.sync.dma_start(out=r, in_=u_raw[(qc, h)][D:D + 1, :])
                    den_sb[(qc, h)] = r

            def emit_recip_bcast(qc):
                for h in range(HL):
                    r = den_sb.pop((qc, h))
                    nc.vector.reciprocal_approx_fast(r, r)
                    if debug and qc == 0 and h == 0:
                        nc.sync.dma_start(out=dbg["d_den0"][:, :], in_=r)
                    bc = sb.tile([D, 512], F32, tag="bcsb", bufs=4, name=f"bc{h}")
                    nc.gpsimd.partition_broadcast(bc, r)
                    if debug and qc == 0 and h == 0:
                        nc.sync.dma_start(out=dbg["d_bc0"][:, :], in_=bc)
                    bc_sb[(qc, h)] = bc

            def emit_norm_mul(qc, hs):
                qsl = slice(qc * QC, (qc + 1) * QC)
                for h in hs:
                    pr, j = divmod(h, 2)
                    if j == 0:
                        nc.vector.tensor_mul(u_pair[pr][0:D, qsl],
                                             u_raw[(qc, h)][0:D, :], bc_sb[(qc, h)])
                    else:
                        stg = sb.tile([D, 512], BF16, tag="ustg", bufs=2,
                                      name=f"ustg{h}")
                        nc.vector.tensor_mul(stg, u_raw[(qc, h)][0:D, :],
                                             bc_sb[(qc, h)])
                        nc.sync.dma_start(out=u_pair[pr][D:2 * D, qsl], in_=stg)
                    del u_raw[(qc, h)], bc_sb[(qc, h)]

            NIT = NQ * KT
            for it in range(NIT):
                qc, kt = divmod(it, KT)
                emit_scores_exp(qc, kt)
                prev = qc - 1
                if prev >= 0:
                    boundary = qc * KT + SKEW
                    if it == boundary:
                        emit_evict(prev)
                        emit_den_dma(prev)
                    elif it == boundary + 1:
                        emit_recip_bcast(prev)
                    elif it == boundary + 2:
                        emit_norm_mul(prev, (0, 1))
                    elif it == boundary + 3:
                        emit_norm_mul(prev, (2, 3))
                if it >= SKEW:
                    emit_pv(*divmod(it - SKEW, KT))
            for it in range(NIT, NIT + SKEW):
                emit_pv(*divmod(it - SKEW, KT))
            emit_evict(NQ - 1)
            emit_den_dma(NQ - 1)
            emit_recip_bcast(NQ - 1)
            emit_norm_mul(NQ - 1, (0, 1, 2, 3))

            if debug:
                nc.sync.dma_start(out=dbg["d_qc0"][:, :], in_=qcont[0])
                nc.sync.dma_start(out=dbg["d_kc0"][:, :], in_=kcont[0])
                nc.sync.dma_start(out=dbg["d_up0"][:, :], in_=u_pair[0])

            # ---------------- phase 3: output projection (partial), K=128
            for g in range(4):
                for i in range(4):
                    st = g * 4 + i
                    y_ps = ps.tile([128, 1024], F32, tag="sc", bufs=2)
                    for ec in range(2):
                        for pr in range(2):
                            nc.tensor.matmul(
                                y_ps[:, ec * 512:(ec + 1) * 512],
                                u_pair[pr][:, st * 128:(st + 1) * 128],
                                wo_sb[:, pr, ec * 512:(ec + 1) * 512],
                                start=(pr == 0), stop=(pr == 1))
                    y_sb = sb.tile([128, E], BF16, tag="ysb", bufs=4,
                                   name=f"ysb{st}")
                    if st % 2 == 0:
                        nc.scalar.copy(y_sb, y_ps)
                    else:
                        nc.vector.tensor_copy(y_sb, y_ps)
                    nc.sync.dma_start(out=ypart[g, :, i * E:(i + 1) * E], in_=y_sb)
    nc.finalize()
    return nc


def make_inputs(x, w_qkv, w_out):
    """Host-side prep: quantize, cast bf16, split/re-layout per core."""
    bf16 = ml_dtypes.bfloat16
    x = np.asarray(x, dtype=np.float32)
    wq_deq = quantize_bits_np(np.asarray(w_qkv, dtype=np.float32))
    wo_deq = quantize_bits_np(np.asarray(w_out, dtype=np.float32))
    cosT, sinT = rope_tables()

    # xt[p, et, s] = x[b].T[et*128+p, s]
    x_t = [np.ascontiguousarray(
        x[b].T.reshape(ET, 128, S).transpose(1, 0, 2)).astype(bf16)
        for b in range(B)]

    in_maps = []
    for c in range(8):
        b, hg = divmod(c, 4)
        heads = [hg * HL + i for i in range(HL)]
        # interleaved q/k col-tiles: ct 0=q d_lo, 1=q d_hi, 2=k d_lo, 3=k d_hi
        # column order within a tile: [h0(32) h1(32) h2(32) h3(32)]
        wqk_t = np.empty((4, E, 128), dtype=np.float32)
        for half in range(2):
            cols = np.concatenate(
                [np.arange(h * D + 32 * half, h * D + 32 * half + 32) for h in heads])
            wqk_t[0 + half] = wq_deq[:, 0 * E + cols]
            wqk_t[2 + half] = wq_deq[:, 1 * E + cols]
        # -> [p, ct, et, c]
        wqk_r = np.ascontiguousarray(
            wqk_t.reshape(4, ET, 128, 128).transpose(2, 0, 1, 3)).astype(bf16)
        vcols = np.concatenate([np.arange(h * D, h * D + D) for h in heads])
        wv_r = np.ascontiguousarray(
            wq_deq[:, 2 * E + vcols].reshape(ET, 128, HL * D)
            .transpose(1, 0, 2)).astype(bf16)
        # wo[p, pr, e]: rows = head (2pr + p//64) of this group, d = p%64
        wo_r = np.empty((128, 2, E), dtype=np.float32)
        for pr in range(2):
            h0 = heads[2 * pr]
            wo_r[:, pr, :] = wo_deq[h0 * D:(h0 + 2) * D, :]
        in_maps.append({
            "xt": x_t[b],
            "wqk": wqk_r, "wv": wv_r,
            "wo": wo_r.astype(bf16),
            "cost": cosT, "sint": sinT,
        })
    return in_maps


_NC_CACHE = {}


def get_nc():
    if "nc" not in _NC_CACHE:
        _NC_CACHE["nc"] = build_kernel()
    return _NC_CACHE["nc"]


def kernel(x, w_qkv, w_out):
    from concourse.bass_utils import run_bass_kernel_spmd
    nc = get_nc()
    in_maps = make_inputs(x, w_qkv, w_out)
    res = run_bass_kernel_spmd(nc, in_maps, list(range(8)))
    out = np.zeros((B, S, E), dtype=np.float32)
    for c in range(8):
        yp = np.asarray(res.results[c]["ypart"], dtype=np.float32)
        # [4, 128, 4, E] -> [S, E]
        y = yp.reshape(4, 128, 4, E).transpose(0, 2, 1, 3).reshape(S, E)
        out[c // 4] += y
    return out
